# revision 28
# baseline (speedup 1.0000x reference)
"""Fused multi-head attention block (B=16, N=1024, C=768, H=12, D=64) for 8
TRN2 NeuronCores. Data-parallel over batch: 2 batches per core, no
collectives. Per-core kernel: qkv matmul -> per-head LayerNorm -> partial
RoPE -> attention (softmax without max-subtraction; denominator fused as a
ones-column in the PV matmul) -> output projection + bias.

Matmul operands are bf16 (PE full rate); accumulation, LayerNorm statistics,
softmax denominators and the final bias-add stay fp32.
"""

import os
import sys

sys.path.insert(0, "/opt/trn_rl_repo")

import numpy as np

import concourse.bass as bass
import concourse.mybir as mybir
import concourse.tile as tile
from concourse import bacc
from concourse.masks import make_identity
from concourse.bass_utils import run_bass_kernel_spmd

F32 = mybir.dt.float32
BF16 = mybir.dt.bfloat16

B_LOC = 2          # batches per core
S = 1024           # sequence length
C = 768            # model dim
H = 12             # heads
D = 64             # head dim
G = 6              # head pairs (2 heads each)
TCH = 8            # 128-token chunks per batch
P_TOK = 1          # num_prefix_tokens
L_TOK = 32         # num_latent_tokens
ROT = S - P_TOK - L_TOK  # 991 rotated tokens
SCALE = D ** -0.5

LAST_RESULT = None


def _bc(ap, dims):
    """Raw broadcast AP: same tensor/offset, explicit [step, count] dims."""
    return bass.AP(tensor=ap.tensor, offset=ap.offset, ap=dims)


def build_nc():
    nc = bacc.Bacc("TRN2", target_bir_lowering=False, debug=False, num_devices=8)

    x_d = nc.declare_dram_parameter("x", [B_LOC * S, C], F32, isOutput=False)
    cos_d = nc.declare_dram_parameter("cos", [ROT, D // 2], F32, isOutput=False)
    sin_d = nc.declare_dram_parameter("sin", [ROT, D // 2], F32, isOutput=False)
    wqkv_d = nc.declare_dram_parameter("w_qkv", [C, 3 * C], F32, isOutput=False)
    wproj_d = nc.declare_dram_parameter("w_proj", [C, C], F32, isOutput=False)
    bproj_d = nc.declare_dram_parameter("b_proj", [C], F32, isOutput=False)
    out_d = nc.declare_dram_parameter("out", [B_LOC * S, C], F32, isOutput=True)

    with tile.TileContext(nc) as tc:
        _build_body(nc, tc, x_d, cos_d, sin_d, wqkv_d, wproj_d, bproj_d, out_d)

    # All ACT functions used here (Exp, Ln, Square, Copy) live together in
    # the natural_log_exp_and_others table set, but the table-load pass
    # assigns each activation the first set containing its function, which
    # alternates exp/ln sets and inserts ~190 table loads (~2.7us each).
    # Present filtered tables (same order/indices) so the shared set is the
    # unique covering choice and the fixpoint pass hoists a single load.
    import concourse.bacc as bacc_mod
    used = {mybir.ActivationFunctionType.Exp, mybir.ActivationFunctionType.Ln,
            mybir.ActivationFunctionType.Square, mybir.ActivationFunctionType.Copy,
            mybir.ActivationFunctionType.Identity}
    orig_gat = bacc_mod.get_activation_tables

    def _gat(arch):
        tabs = orig_gat(arch)
        out = {}
        for name, fns in tabs.items():
            if name == "natural_log_exp_and_others":
                out[name] = fns
            else:
                out[name] = fns - used
        return out

    bacc_mod.get_activation_tables = _gat
    try:
        nc.compile()
    finally:
        bacc_mod.get_activation_tables = orig_gat
    return nc


def _build_body(nc, tc, x_d, cos_d, sin_d, wqkv_d, wproj_d, bproj_d, out_d):
    from contextlib import ExitStack

    ctx = ExitStack()
    with ctx:
        singles = ctx.enter_context(tc.tile_pool(name="singles", bufs=1))
        stage_pool = ctx.enter_context(tc.tile_pool(name="stage", bufs=2))
        xin_pool = ctx.enter_context(tc.tile_pool(name="xin", bufs=2))
        xt_pool = ctx.enter_context(tc.tile_pool(name="xt", bufs=2))
        at_pool = ctx.enter_context(tc.tile_pool(name="at", bufs=2))
        qt_pool = ctx.enter_context(tc.tile_pool(name="qt", bufs=2))
        kt_pool = ctx.enter_context(tc.tile_pool(name="kt", bufs=2))
        v_pool = ctx.enter_context(tc.tile_pool(name="v", bufs=2))
        ln_pool = ctx.enter_context(tc.tile_pool(name="ln", bufs=3))
        st_pool = ctx.enter_context(tc.tile_pool(name="st", bufs=4))
        p_pool = ctx.enter_context(tc.tile_pool(name="p", bufs=3))
        rb_pool = ctx.enter_context(tc.tile_pool(name="rb", bufs=2))
        ob_pool = ctx.enter_context(tc.tile_pool(name="ob", bufs=3))

        qkv_ps = ctx.enter_context(tc.tile_pool(name="qkvps", bufs=2, space="PSUM"))
        t_ps = ctx.enter_context(tc.tile_pool(name="tps", bufs=2, space="PSUM"))
        sc_ps = ctx.enter_context(tc.tile_pool(name="scps", bufs=2, space="PSUM"))
        o_ps = ctx.enter_context(tc.tile_pool(name="ops", bufs=2, space="PSUM"))

        # ---- one-time setup ----
        ident = singles.tile([128, 128], BF16)
        make_identity(nc, ident)
        identf = singles.tile([128, 128], F32)
        make_identity(nc, identf)

        eps_t = singles.tile([128, 1], F32)
        nc.vector.memset(eps_t, 1e-5)

        # bias broadcast across partitions via DMA replication from DRAM
        bias_bc = singles.tile([128, C], F32)
        bp = bproj_d[:]
        nc.gpsimd.dma_start(out=bias_bc, in_=_bc(bp, [[0, 128]] + list(bp.ap)))

        # weights: gpsimd DMAs cast fp32 -> bf16 directly (w_qkv first — it
        # gates the first matmuls; w_proj later, it is needed only at proj)
        wq_all = singles.tile([128, G, 3 * C], BF16)
        wp_all = singles.tile([128, G, C], BF16)
        for cc in range(G):
            nc.gpsimd.dma_start(out=wq_all[:, cc, :],
                                in_=wqkv_d[cc * 128:(cc + 1) * 128, :])

        # cos/sin per token-chunk (position p of chunk ch holds angle row
        # ch*128 + p - 1; token 0 and tokens >= 992 are not rotated)
        cs_t, sn_t = [], []
        for ch in range(TCH):
            ct = singles.tile([128, 32], BF16, tag=f"cos{ch}")
            st = singles.tile([128, 32], BF16, tag=f"sin{ch}")
            if ch == 0:
                # partition 0 (unrotated token 0) gets cos=sin=0 so the
                # full-range rope op stays finite; overwritten by copy after
                nc.vector.memset(ct[0:1, :], 0.0)
                nc.vector.memset(st[0:1, :], 0.0)
                nc.gpsimd.dma_start(out=ct[1:128, :], in_=cos_d[0:127, :])
                nc.gpsimd.dma_start(out=st[1:128, :], in_=sin_d[0:127, :])
            elif ch == 7:
                nc.gpsimd.dma_start(out=ct[0:96, :], in_=cos_d[895:991, :])
                nc.gpsimd.dma_start(out=st[0:96, :], in_=sin_d[895:991, :])
            else:
                nc.gpsimd.dma_start(out=ct, in_=cos_d[ch * 128 - 1: ch * 128 + 127, :])
                nc.gpsimd.dma_start(out=st, in_=sin_d[ch * 128 - 1: ch * 128 + 127, :])
            cs_t.append(ct)
            sn_t.append(st)

        # softmax-denominator tiles + per-pair broadcast-selector masks
        dn = singles.tile([128, S], F32)     # raw denominators (rows 0..11)
        dnb = singles.tile([128, S], BF16)   # reciprocals, bcast-matmul rhs
        nc.gpsimd.memset(dn, 0.0)
        nc.gpsimd.memset(dnb, 0.0)
        masks = []
        for g in range(G):
            # mask[k, h*64+j] = 1 iff k == 2g + h
            mk = singles.tile([128, 128], BF16, tag=f"mask{g}")
            nc.gpsimd.memset(mk, 0.0)
            mk3 = mk[:].rearrange("p (h j) -> p h j", h=2)
            nc.gpsimd.affine_select(
                out=mk3, in_=mk3,
                compare_op=mybir.AluOpType.not_equal,
                fill=1.0, base=-2 * g,
                pattern=[[-1, 2], [0, 64]],
                channel_multiplier=1)
            masks.append(mk)

        for cc in range(G):
            nc.gpsimd.dma_start(out=wp_all[:, cc, :],
                                in_=wproj_d[cc * 128:(cc + 1) * 128, :])

        for b in range(B_LOC):
            # ---- x^T (bf16) for this batch: [128(c), cc, t] ----
            xt_b = xt_pool.tile([128, G, S], BF16, tag="xt")
            for ch in range(TCH):
                xin = xin_pool.tile([128, C], F32, tag="xin")
                nc.sync.dma_start(
                    out=xin, in_=x_d[b * S + ch * 128: b * S + (ch + 1) * 128, :])
                for cc in range(G):
                    tp = t_ps.tile([128, 128], F32, tag="tps")
                    nc.tensor.transpose(tp, xin[:, cc * 128:(cc + 1) * 128], identf)
                    nc.vector.tensor_copy(xt_b[:, cc, ch * 128:(ch + 1) * 128], tp)

            at_b = at_pool.tile([128, G, S], BF16, tag="at")

            for g in range(G):
                qt = qt_pool.tile([128, S], BF16, tag="qt")
                kt = kt_pool.tile([128, S], BF16, tag="kt")
                vg = v_pool.tile([128, TCH, 2, 65], BF16, tag="vg")
                nc.gpsimd.memset(vg[:, :, :, 64:65], 1.0)

                for ch in range(TCH):
                    # qkv matmul for this (batch, pair, chunk): [128t, 384]
                    qps = qkv_ps.tile([128, 384], F32, tag="qkv")
                    for cc in range(G):
                        # rhs: cols part*768 + g*128 + j for part in 0..2
                        rhs = wq_all[:, cc, :].rearrange(
                            "p (t g j) -> p t g j", t=3, j=128)[:, :, g, :]
                        nc.tensor.matmul(
                            qps,
                            lhsT=xt_b[:, cc, ch * 128:(ch + 1) * 128],
                            rhs=rhs,
                            start=(cc == 0), stop=(cc == G - 1))

                    # ---- layernorm stats for the 4 q/k groups ----
                    qk3 = qps[:, 0:256].rearrange("p (g d) -> p g d", d=64)
                    ssum = st_pool.tile([128, 4], F32, tag="ssum")
                    nc.vector.reduce_sum(ssum, qk3, axis=mybir.AxisListType.X)
                    sq = ln_pool.tile([128, 256], F32, tag="sq")
                    nc.scalar.square(sq, qps[:, 0:256])
                    ssq = st_pool.tile([128, 4], F32, tag="ssq")
                    nc.vector.reduce_sum(
                        ssq, sq.rearrange("p (g d) -> p g d", d=64),
                        axis=mybir.AxisListType.X)

                    # ---- v eviction (+ ones column pre-set above) ----
                    nc.scalar.copy(
                        vg[:, ch, :, 0:64],
                        qps[:, 256:384].rearrange("p (h d) -> p h d", d=64))

                    qk3 = qps[:, 0:256].rearrange("p (g d) -> p g d", d=64)
                    mu = st_pool.tile([128, 4], F32, tag="mu")
                    nc.scalar.mul(mu, ssum, 1.0 / 64)
                    msq = st_pool.tile([128, 4], F32, tag="msq")
                    nc.vector.tensor_mul(msq, mu, mu)
                    rs = st_pool.tile([128, 4], F32, tag="rs")
                    nc.vector.scalar_tensor_tensor(
                        out=rs, in0=ssq, scalar=1.0 / 64, in1=msq,
                        op0=mybir.AluOpType.mult, op1=mybir.AluOpType.subtract)
                    # rsqrt via ln+exp: keeps ACT on the natural_log_exp table
                    # set (same set as softmax exp) — a Sqrt call would force a
                    # ~2.7us ACT table-set switch per use
                    nc.scalar.activation(rs, rs, mybir.ActivationFunctionType.Ln,
                                         bias=eps_t)
                    nc.scalar.activation(rs, rs, mybir.ActivationFunctionType.Exp,
                                         scale=-0.5)
                    qk_ln = ln_pool.tile([128, 256], BF16, tag="qkln")
                    mu_b = _bc(mu[:], [mu[:].ap[0], [1, 4], [0, 64]])
                    rs_b = _bc(rs[:], [rs[:].ap[0], [1, 4], [0, 64]])
                    qkl3 = qk_ln[:].rearrange("p (g d) -> p g d", d=64)
                    nc.vector.tensor_sub(qkl3, qk3, mu_b)
                    nc.vector.tensor_mul(qkl3, qkl3, rs_b)

                    # ---- partial RoPE: q on DVE, k on Pool, concurrently ----
                    if ch == 0:
                        pr = (0, 128)   # full range; row 0 fixed by copy below
                        cp = (0, 1)
                    elif ch == 7:
                        pr = (0, 96)
                        cp = (96, 128)
                    else:
                        pr = (0, 128)
                        cp = None
                    q_rot = ln_pool.tile([128, 128], BF16, tag="qrot")
                    k_rot = ln_pool.tile([128, 128], BF16, tag="krot")
                    p0, p1 = pr
                    for peng, base, rot_t, tg in (
                            (nc.vector, 0, q_rot, "q"), (nc.gpsimd, 128, k_rot, "k")):
                        t_cc = ln_pool.tile([128, 128], BF16, tag=f"tcc{tg}")
                        t_ss = ln_pool.tile([128, 128], BF16, tag=f"tss{tg}")
                        srcv = qk_ln[p0:p1, base:base + 128].rearrange(
                            "p (h j t) -> p h j t", j=32, t=2)
                        ctile = cs_t[ch][p0:p1, :]
                        stile = sn_t[ch][p0:p1, :]
                        cos4 = _bc(ctile, [ctile.ap[0], [0, 2], [1, 32], [0, 2]])
                        sin4 = _bc(stile, [stile.ap[0], [0, 2], [1, 32], [0, 2]])
                        ccv = t_cc[p0:p1, :].rearrange("p (h j t) -> p h j t", j=32, t=2)
                        ssv = t_ss[p0:p1, :].rearrange("p (h j t) -> p h j t", j=32, t=2)
                        rot = rot_t[p0:p1, :].rearrange("p (h j t) -> p h j t", j=32, t=2)
                        peng.tensor_mul(ccv, srcv, cos4)
                        peng.tensor_mul(ssv, srcv, sin4)
                        peng.tensor_sub(rot[:, :, :, 0:1], ccv[:, :, :, 0:1],
                                        ssv[:, :, :, 1:2])
                        peng.tensor_add(rot[:, :, :, 1:2], ssv[:, :, :, 0:1],
                                        ccv[:, :, :, 1:2])
                    if cp is not None:
                        c0, c1 = cp
                        nc.scalar.copy(q_rot[c0:c1, :], qk_ln[c0:c1, 0:128])
                        nc.scalar.copy(k_rot[c0:c1, :], qk_ln[c0:c1, 128:256])

                    # ---- transpose q/k pair blocks to feature-major ----
                    tpq = t_ps.tile([128, 128], BF16, tag="tps")
                    nc.tensor.transpose(tpq, q_rot, ident)
                    # q scale folded into the eviction
                    nc.vector.tensor_scalar_mul(
                        out=qt[:, ch * 128:(ch + 1) * 128], in0=tpq, scalar1=SCALE)
                    tpk = t_ps.tile([128, 128], BF16, tag="tps")
                    nc.tensor.transpose(tpk, k_rot, ident)
                    nc.vector.tensor_copy(kt[:, ch * 128:(ch + 1) * 128], tpk)

                # ---- attention for the two heads of this pair ----
                for hl in range(2):
                    for tqh in range(2):
                        ops = o_ps.tile([65, 512], F32, tag="ops")
                        for tk in range(TCH):
                            scps = sc_ps.tile([128, 512], F32, tag="scps")
                            # K=64 contraction: head hl lives in partition
                            # rows hl*64..hl*64+63 of kt and qt
                            nc.tensor.matmul(
                                scps,
                                lhsT=kt[hl * 64:(hl + 1) * 64,
                                        tk * 128:(tk + 1) * 128],
                                rhs=qt[hl * 64:(hl + 1) * 64,
                                       tqh * 512:(tqh + 1) * 512],
                                start=True, stop=True)
                            pt = p_pool.tile([128, 512], BF16, tag="pt")
                            nc.scalar.activation(pt, scps,
                                                 mybir.ActivationFunctionType.Exp)
                            nc.tensor.matmul(
                                ops,
                                lhsT=vg[:, tk, hl, :],
                                rhs=pt[:],
                                start=(tk == 0), stop=(tk == TCH - 1))
                        nc.vector.tensor_copy(
                            at_b[hl * 64:(hl + 1) * 64, g, tqh * 512:(tqh + 1) * 512],
                            ops[0:64, :])
                        # denominator row -> partition-0 tile -> DMA into dn
                        # (compute engines need aligned partition bases; DMA
                        # can write any partition)
                        drow = st_pool.tile([1, 512], F32, tag="drow")
                        nc.vector.tensor_copy(drow, ops[64:65, :])
                        nc.sync.dma_start(
                            out=dn[2 * g + hl: 2 * g + hl + 1,
                                   tqh * 512:(tqh + 1) * 512],
                            in_=drow)

            # ---- normalize by softmax denominators ----
            nc.vector.reciprocal(dn[0:12, :], dn[0:12, :])
            nc.vector.tensor_copy(dnb[0:12, :], dn[0:12, :])
            for g in range(G):
                for tqh in range(2):
                    bps = sc_ps.tile([128, 512], F32, tag="scps")
                    nc.tensor.matmul(
                        bps, lhsT=masks[g][:],
                        rhs=dnb[:, tqh * 512:(tqh + 1) * 512],
                        start=True, stop=True)
                    sl = at_b[:, g, tqh * 512:(tqh + 1) * 512]
                    nc.vector.tensor_mul(sl, sl, bps)

            # ---- output projection ----
            for ch in range(TCH):
                for fp in range(2):
                    pps = qkv_ps.tile([128, 384], F32, tag="qkv")
                    for cc in range(G):
                        nc.tensor.matmul(
                            pps,
                            lhsT=at_b[:, cc, ch * 128:(ch + 1) * 128],
                            rhs=wp_all[:, cc, fp * 384:(fp + 1) * 384],
                            start=(cc == 0), stop=(cc == G - 1))
                    ob = ob_pool.tile([128, 384], F32, tag="ob")
                    nc.vector.tensor_add(ob, pps, bias_bc[:, fp * 384:(fp + 1) * 384])
                    nc.sync.dma_start(
                        out=out_d[b * S + ch * 128: b * S + (ch + 1) * 128,
                                  fp * 384:(fp + 1) * 384],
                        in_=ob)


_NC_CACHE = None


def kernel(**inputs):
    global LAST_RESULT, _NC_CACHE
    x = np.ascontiguousarray(np.asarray(inputs["x"], dtype=np.float32))
    cos = np.ascontiguousarray(np.asarray(inputs["cos"], dtype=np.float32))
    sin = np.ascontiguousarray(np.asarray(inputs["sin"], dtype=np.float32))
    w_qkv = np.ascontiguousarray(np.asarray(inputs["w_qkv"], dtype=np.float32))
    w_proj = np.ascontiguousarray(np.asarray(inputs["w_proj"], dtype=np.float32))
    b_proj = np.ascontiguousarray(np.asarray(inputs["b_proj"], dtype=np.float32))

    if _NC_CACHE is None:
        _NC_CACHE = build_nc()
    nc = _NC_CACHE

    n_cores = 8
    in_maps = []
    for c in range(n_cores):
        in_maps.append({
            "x": x[B_LOC * c: B_LOC * (c + 1)].reshape(B_LOC * S, C),
            "cos": cos, "sin": sin,
            "w_qkv": w_qkv, "w_proj": w_proj, "b_proj": b_proj,
        })

    res = run_bass_kernel_spmd(
        nc, in_maps, core_ids=list(range(n_cores)),
        trace=bool(os.environ.get("BASS_TRACE")),
    )
    LAST_RESULT = res
    out = np.concatenate(
        [res.results[c]["out"].reshape(B_LOC, S, C) for c in range(n_cores)], axis=0)
    return out.astype(np.float32)
